# revision 16
# baseline (speedup 1.0000x reference)
# MoE top-2 routing kernel for Trainium2, 8 NeuronCores, data-parallel over batch.
# Self-contained: hardcodes shapes B=8, S=2048, D=1024, E=8, TOP_K=2.
#
# Math: out = sum_e w_e * (X @ We[e]^T + be[e]) @ Wo^T + bo   (w_e = masked top-2
# softmax weights). Since w_e is a per-token scalar, fold Wo into each expert on
# the host:  G_e = We[e]^T @ Wo^T  (weight-only preprocessing), so the device
# computes  out = sum_e w_e * (X @ G_e) + [w|1] @ [be@Wo^T; bo]  with no
# token dispatch/gather at all.
import numpy as np

B, S, D, E = 8, 2048, 1024, 8
TOPK = 2
P = 128
NKT = D // P   # 8 contraction tiles
NT = S // P    # 16 token tiles
H2 = D // 512  # psum-bank halves of the output dim


def build_kernel(reps=1):
    """Build the per-core Bass module. reps>1 wraps the whole body in a
    hardware loop (identical iterations) for steady-state timing."""
    import concourse.bacc as bacc
    import concourse.tile as tile
    import concourse.mybir as mybir
    from concourse.masks import make_identity
    import contextlib

    dt = mybir.dt
    fp32 = dt.float32
    bf16 = dt.bfloat16
    Copy = mybir.ActivationFunctionType.Copy

    nc = bacc.Bacc(None, target_bir_lowering=False, debug=False)

    XTld = nc.declare_dram_parameter("XTl", [P, NKT, S], bf16, isOutput=False)
    XTbd = nc.declare_dram_parameter("XTb", [P, NKT, S], bf16, isOutput=False)
    Gd = nc.declare_dram_parameter("G", [P, E, NKT, D], bf16, isOutput=False)
    WrTd = nc.declare_dram_parameter("WrT", [P, NKT, 2, E], bf16, isOutput=False)
    brd = nc.declare_dram_parameter("br", [E, 1], fp32, isOutput=False)
    be9d = nc.declare_dram_parameter("be9p", [E + 1, D], bf16, isOutput=False)
    outd = nc.declare_dram_parameter("out", [S, D], bf16, isOutput=True)
    CW = 256               # router chunk width (>=256 keeps f32r at 1 cyc/row)
    NC_CH = S // CW

    with tile.TileContext(nc) as tc:
        ctx = contextlib.ExitStack()
        with ctx:
            const_p = ctx.enter_context(tc.tile_pool(name="const", bufs=1))
            w_p = ctx.enter_context(tc.tile_pool(name="wts", bufs=1))
            xtf_p = ctx.enter_context(tc.tile_pool(name="xtf", bufs=3))
            sm_p = ctx.enter_context(tc.tile_pool(name="sm", bufs=4))
            acc_p = ctx.enter_context(tc.tile_pool(name="acc", bufs=NT))
            g_p = ctx.enter_context(tc.tile_pool(name="gp", bufs=2))
            psA_p = ctx.enter_context(tc.tile_pool(name="psA", bufs=3, space="PSUM"))
            ptr_p = ctx.enter_context(tc.tile_pool(name="ptr", bufs=2, space="PSUM"))

            # constants (identity matrices) — true compile-time constants
            ID = const_p.tile([P, P], fp32)
            make_identity(nc, ID[:])
            IDb = const_p.tile([P, P], bf16)
            nc.vector.tensor_copy(out=IDb[:], in_=ID[:])

            def body():
                # ---- input / weight loads (per execution) ----
                WrTs = w_p.tile([P, NKT, 2, E], bf16, tag="wr")
                nc.sync.dma_start(out=WrTs[:], in_=WrTd[:])
                brS = w_p.tile([E, 1], fp32, tag="br")
                nc.sync.dma_start(out=brS[:], in_=brd[:])
                g0 = g_p.tile([P, NKT, D], bf16, tag="g")
                nc.sync.dma_start(out=g0[:], in_=Gd[:, 0, :, :])
                be9p = w_p.tile([E + 1, D], bf16, tag="be9")
                nc.sync.dma_start(out=be9p[:], in_=be9d[:])
                XTb = w_p.tile([P, NKT, S], bf16, tag="xtb")
                XCH = 512
                for xc in range(S // XCH):
                    nc.sync.dma_start(
                        out=XTb[:, :, xc * XCH:(xc + 1) * XCH],
                        in_=XTbd[:, :, xc * XCH:(xc + 1) * XCH])

                # ---- router pre-pass: top-2 masked weights for all tiles ----
                # w9_all[:, t*9 : t*9+9] = [masked top-2 softmax weights | 1]
                w9_all = w_p.tile([P, NT * (E + 1)], fp32, tag="w9a")
                for c in range(NC_CH):
                    csl = slice(c * CW, (c + 1) * CW)
                    xtl = xtf_p.tile([P, NKT, CW], bf16, tag="xtf")
                    nc.sync.dma_start(out=xtl[:], in_=XTld[:, :, csl])
                    ltp = ptr_p.tile([E, CW], fp32, space="PSUM", tag="tr")
                    for kt in range(NKT):
                        nc.tensor.matmul(
                            out=ltp[:], lhsT=WrTs[:, kt, 0, :],
                            rhs=XTb[:, kt, csl], start=(kt == 0), stop=False)
                    for kt in range(NKT):
                        nc.tensor.matmul(
                            out=ltp[:], lhsT=WrTs[:, kt, 1, :],
                            rhs=XTb[:, kt, csl], start=False, stop=False)
                    for kt in range(NKT):
                        nc.tensor.matmul(
                            out=ltp[:], lhsT=WrTs[:, kt, 0, :],
                            rhs=xtl[:, kt, :], start=False,
                            stop=(kt == NKT - 1))
                    LTc = sm_p.tile([E, CW], fp32, tag="ltc")
                    nc.vector.tensor_scalar(
                        out=LTc[:], in0=ltp[:], scalar1=brS[:, 0:1], scalar2=None,
                        op0=mybir.AluOpType.add)
                    for u in range(CW // P):
                        t = c * (CW // P) + u
                        lp = ptr_p.tile([P, E], fp32, space="PSUM", tag="tr")
                        nc.tensor.transpose(
                            out=lp[:], in_=LTc[:, u * P:(u + 1) * P],
                            identity=ID[:E, :E])
                        Ls = sm_p.tile([P, E], fp32, tag="ls")
                        nc.vector.tensor_copy(out=Ls[:], in_=lp[:])
                        mneg = sm_p.tile([P, 1], fp32, tag="mneg")
                        nc.vector.tensor_reduce(
                            out=mneg[:], in_=Ls[:], axis=mybir.AxisListType.X,
                            op=mybir.AluOpType.max, negate=True)
                        Eexp = sm_p.tile([P, E], fp32, tag="eexp")
                        Zs = sm_p.tile([P, 1], fp32, tag="zs")
                        nc.scalar.activation(
                            out=Eexp[:], in_=Ls[:],
                            func=mybir.ActivationFunctionType.Exp,
                            bias=mneg[:, 0:1], scale=1.0, accum_out=Zs[:, 0:1])
                        rZ = sm_p.tile([P, 1], fp32, tag="rz")
                        nc.vector.reciprocal(out=rZ[:], in_=Zs[:])
                        Wsm = sm_p.tile([P, E], fp32, tag="wsm")
                        nc.vector.tensor_scalar_mul(Wsm[:], Eexp[:], rZ[:, 0:1])
                        Wm8 = sm_p.tile([P, E], fp32, tag="wm8")
                        nc.vector.max(out=Wm8[:], in_=Wsm[:])
                        mr8 = sm_p.tile([P, E], fp32, tag="mr8")
                        nc.vector.tensor_copy(out=mr8[:], in_=Wm8[:])
                        nc.vector.memset(mr8[:, TOPK:], -1.0)
                        Wz = sm_p.tile([P, E], fp32, tag="wz")
                        nc.vector.match_replace(
                            out=Wz[:], in_to_replace=mr8[:], in_values=Wsm[:],
                            imm_value=0.0)
                        w9c = w9_all[:, t * (E + 1):(t + 1) * (E + 1)]
                        nc.vector.tensor_sub(
                            out=w9c[:, :E], in0=Wsm[:], in1=Wz[:])
                        nc.vector.memset(w9c[:, E:], 1.0)

                # ---- bias phase: ACC_t = [w|1] @ [be@Wo^T; bo] ----
                ACCs = []
                for t in range(NT):
                    w9c = w9_all[:, t * (E + 1):(t + 1) * (E + 1)]
                    w9tp = ptr_p.tile([E + 1, P], fp32, space="PSUM", tag="tr")
                    nc.tensor.transpose(out=w9tp[:], in_=w9c, identity=ID[:])
                    w9t = sm_p.tile([E + 1, P], bf16, tag="w9t")
                    nc.vector.tensor_copy(out=w9t[:], in_=w9tp[:])
                    psB = psA_p.tile([P, D], fp32, space="PSUM", tag="a")
                    for h2 in range(H2):
                        hsl = slice(h2 * 512, (h2 + 1) * 512)
                        nc.tensor.matmul(
                            out=psB[:, hsl], lhsT=w9t[:], rhs=be9p[:, hsl],
                            start=True, stop=True)
                    ACC = acc_p.tile([P, D], fp32, tag="acc")
                    nc.scalar.activation(out=ACC[:], in_=psB[:], func=Copy)
                    ACCs.append(ACC)

                # ---- experts outer (G double-buffered), tiles inner ----
                for e in range(E):
                    if e == 0:
                        g = g0
                    else:
                        g = g_p.tile([P, NKT, D], bf16, tag="g")
                        nc.sync.dma_start(out=g[:], in_=Gd[:, e, :, :])
                    for t in range(NT):
                        tsl = slice(t * P, (t + 1) * P)
                        w9c = w9_all[:, t * (E + 1):(t + 1) * (E + 1)]
                        psA = psA_p.tile([P, D], fp32, space="PSUM", tag="a")
                        for kt in range(NKT):
                            for h2 in range(H2):
                                hsl = slice(h2 * 512, (h2 + 1) * 512)
                                nc.tensor.matmul(
                                    out=psA[:, hsl], lhsT=XTb[:, kt, tsl],
                                    rhs=g[:, kt, hsl],
                                    start=(kt == 0), stop=(kt == NKT - 1))
                        nc.vector.scalar_tensor_tensor(
                            out=ACCs[t][:], in0=psA[:], scalar=w9c[:, e:e + 1],
                            in1=ACCs[t][:], op0=mybir.AluOpType.mult,
                            op1=mybir.AluOpType.add)
                        if e == E - 1:
                            # cast-to-bf16 during DMA (SWDGE)
                            nc.gpsimd.dma_start(out=outd[tsl, :],
                                                in_=ACCs[t][:])

            if reps == 1:
                body()
            else:
                with tc.For_i(0, reps, 1):
                    body()

    nc.compile()
    return nc


_NC_CACHE = {}


def _get_nc(reps=1):
    if reps not in _NC_CACHE:
        _NC_CACHE[reps] = build_kernel(reps)
    return _NC_CACHE[reps]


def make_in_maps(X, We, be, Wr, br, Wo, bo):
    import ml_dtypes
    bf = ml_dtypes.bfloat16
    X = np.asarray(X, np.float32)
    We = np.asarray(We, np.float32)
    Wo = np.asarray(Wo, np.float32)
    be = np.asarray(be, np.float32)
    bo = np.asarray(bo, np.float32)
    Wr = np.asarray(Wr, np.float32)
    br = np.asarray(br, np.float32)

    # G_e = We[e]^T @ Wo^T = (Wo @ We[e])^T, device layout [P, E, NKT, D]
    M = np.matmul(Wo, We)                      # [E, D(out o), D(in d)]
    G = M.transpose(0, 2, 1)                   # [E, d, o]
    Gdev = np.ascontiguousarray(
        G.reshape(E, NKT, P, D).transpose(2, 0, 1, 3)).astype(bf)
    be9p = np.concatenate(
        [be @ Wo.T, bo.reshape(1, D)], axis=0).astype(bf)  # [E+1, D]
    WrTf = np.ascontiguousarray(
        Wr.T.reshape(NKT, P, E).transpose(1, 0, 2)).astype(np.float32)
    WrTh = WrTf.astype(bf)
    WrTl = (WrTf - WrTh.astype(np.float32)).astype(bf)
    WrTdev = np.ascontiguousarray(np.stack([WrTh, WrTl], axis=2))
    brC = np.ascontiguousarray(br.reshape(E, 1))

    maps = []
    for c in range(B):
        XT = np.ascontiguousarray(X[c].T)      # [D, S]
        XTdev = np.ascontiguousarray(
            XT.reshape(NKT, P, S).transpose(1, 0, 2))     # [P, NKT, S]
        XTh = XTdev.astype(bf)
        XTl = (XTdev - XTh.astype(np.float32)).astype(bf)
        maps.append({
            "XTl": XTl,
            "XTb": XTh,
            "G": Gdev,
            "WrT": WrTdev,
            "br": brC,
            "be9p": be9p,
        })
    return maps


def kernel(X, We, be, Wr, br, Wo, bo):
    from concourse.bass_utils import run_bass_kernel_spmd
    nc = _get_nc()
    in_maps = make_in_maps(X, We, be, Wr, br, Wo, bo)
    res = run_bass_kernel_spmd(nc, in_maps, list(range(B)))
    out = np.stack([np.asarray(res.results[c]["out"]) for c in range(B)], axis=0)
    return out.astype(np.float32)
